# revision 2
# baseline (speedup 1.0000x reference)
"""Trainium2 kernel for nn_MultiHeadCrossAttention_70952859730436.

The reference computes
    q = Wq x1 + bq ; k = Wk x2 + bk ; v = Wv x3 + bv          (1x1 convs)
    qk = q k^T / sqrt(32) ; attn = softmax(qk, -1)
    out_v = einsum('bhqk,bhvd->bhqd', attn, v)                 # sums over BOTH k and v
    out = Wo out_v + bo + x4

The einsum contracts `k` and `v` independently, so
    out_v[b,h,q,d] = (sum_k attn[b,h,q,k]) * (sum_v v[b,h,v,d]) = 1 * sum_v v[b,h,v,d]
(softmax rows sum to one).  The attention block therefore collapses to a
per-head channel sum of v, broadcast back to all 32 channels of the head:
    out = Wo (S (Wv x3 + bv)) + bo + x4 = (Wo S Wv) x3 + (Wo S bv + bo) + x4
with S the block matrix S[i,j] = [i//32 == j//32].  q, k, x1, x2 are unused.

The device kernel computes out = W_eff @ x3 + b_eff + x4 per spatial position:
a [256,256] x [256, B*H*W] matmul + bias + residual, data-parallel over batch
(2 batches per core across 8 cores).
"""

import os

import numpy as np

B, C, H, W = 16, 256, 64, 64
HW = H * W  # 4096
RD, HEADS = 256, 8
RDH = RD // HEADS  # 32
NCORES = 8
BPC = B // NCORES  # batches per core = 2

CT = 2048  # spatial column tile (per-partition bytes: 8 KiB)
NCT = HW // CT  # 2 column tiles per batch
PB = 512  # psum free-dim chunk (one PSUM bank at fp32)
NPB = CT // PB  # 4

_CACHE = {}

# Populated with the NEFF execution time (ns) of the slowest core after each
# traced run; test.py reads this.
LAST_EXEC_TIME_NS = None


def _build_program():
    import concourse.mybir as mybir
    import concourse.tile as tile
    from concourse import bacc

    nc = bacc.Bacc(
        "TRN2",
        target_bir_lowering=False,
        debug=False,
        num_devices=NCORES,
    )
    f32 = mybir.dt.float32

    x = nc.dram_tensor("x", (BPC, C, HW), f32, kind="ExternalInput").ap()
    r = nc.dram_tensor("r", (BPC, C, HW), f32, kind="ExternalInput").ap()
    wT = nc.dram_tensor("wT", (C, C), f32, kind="ExternalInput").ap()  # [c_in, c_out]
    bvec = nc.dram_tensor("bvec", (C, 1), f32, kind="ExternalInput").ap()
    out = nc.dram_tensor("out", (BPC, C, HW), f32, kind="ExternalOutput").ap()

    act_ident = mybir.ActivationFunctionType.Identity

    with tile.TileContext(nc) as tc:
        with (
            tc.tile_pool(name="const", bufs=1) as cpool,
            tc.tile_pool(name="xp", bufs=3) as xpool,
            tc.tile_pool(name="rp", bufs=3) as rpool,
            tc.tile_pool(name="op", bufs=3) as opool,
            tc.tile_pool(name="ps", bufs=8, space="PSUM") as pspool,
        ):
            # Stationary weights: lhsT[k][m] = W_eff.T[k-block, m-block]
            w_tiles = {}
            for k in range(2):
                for m in range(2):
                    wt = cpool.tile([128, 128], f32, name=f"w{k}{m}")
                    nc.sync.dma_start(
                        wt[:], wT[k * 128 : (k + 1) * 128, m * 128 : (m + 1) * 128]
                    )
                    w_tiles[k, m] = wt
            b_tiles = []
            for m in range(2):
                bt = cpool.tile([128, 1], f32, name=f"b{m}")
                nc.sync.dma_start(bt[:], bvec[m * 128 : (m + 1) * 128, :])
                b_tiles.append(bt)

            for b in range(BPC):
                for ct in range(NCT):
                    csl = slice(ct * CT, (ct + 1) * CT)
                    xt = xpool.tile([128, 2 * CT], f32, name="xt")
                    rt = rpool.tile([128, 2 * CT], f32, name="rt")
                    ot = opool.tile([128, 2 * CT], f32, name="ot")
                    for k in range(2):
                        nc.sync.dma_start(
                            xt[:, k * CT : (k + 1) * CT],
                            x[b, k * 128 : (k + 1) * 128, csl],
                        )
                    for m in range(2):
                        nc.sync.dma_start(
                            rt[:, m * CT : (m + 1) * CT],
                            r[b, m * 128 : (m + 1) * 128, csl],
                        )
                    for m in range(2):
                        for nb in range(NPB):
                            osl = slice(m * CT + nb * PB, m * CT + (nb + 1) * PB)
                            ps = pspool.tile([128, PB], f32, name="ps")
                            for k in range(2):
                                nc.tensor.matmul(
                                    ps[:],
                                    w_tiles[k, m][:],
                                    xt[:, k * CT + nb * PB : k * CT + (nb + 1) * PB],
                                    start=(k == 0),
                                    stop=(k == 1),
                                )
                            # out = psum + b_eff (per-partition bias), then + x4
                            nc.scalar.activation(
                                ot[:, osl], ps[:], act_ident, bias=b_tiles[m][:]
                            )
                            nc.vector.tensor_add(ot[:, osl], ot[:, osl], rt[:, osl])
                    for m in range(2):
                        nc.sync.dma_start(
                            out[b, m * 128 : (m + 1) * 128, csl],
                            ot[:, m * CT : (m + 1) * CT],
                        )

    nc.compile()
    return nc


def _get_program():
    if "nc" not in _CACHE:
        _CACHE["nc"] = _build_program()
    return _CACHE["nc"]


def _fuse_weights(Wv, bv, Wo, bo):
    # W_eff = Wo @ S @ Wv,  b_eff = Wo @ S @ bv + bo  (S = per-head broadcast-sum)
    Wv = np.asarray(Wv, np.float64)
    bv = np.asarray(bv, np.float64)
    Wo = np.asarray(Wo, np.float64)
    bo = np.asarray(bo, np.float64)
    head_sum_W = Wv.reshape(HEADS, RDH, C).sum(axis=1)  # [8, 256]
    SWv = np.repeat(head_sum_W, RDH, axis=0)  # [256, 256]
    head_sum_b = bv.reshape(HEADS, RDH).sum(axis=1)  # [8]
    Sbv = np.repeat(head_sum_b, RDH)  # [256]
    W_eff = Wo @ SWv  # [C, C]
    b_eff = Wo @ Sbv + bo  # [C]
    return W_eff.astype(np.float32), b_eff.astype(np.float32)


def kernel(x1, x2, x3, x4, Wq, bq, Wk, bk, Wv, bv, Wo, bo):
    global LAST_EXEC_TIME_NS
    from concourse import bass_utils

    W_eff, b_eff = _fuse_weights(Wv, bv, Wo, bo)
    wT = np.ascontiguousarray(W_eff.T)  # [c_in, c_out]
    bvec = np.ascontiguousarray(b_eff.reshape(C, 1))

    x3 = np.ascontiguousarray(np.asarray(x3, np.float32).reshape(B, C, HW))
    x4 = np.ascontiguousarray(np.asarray(x4, np.float32).reshape(B, C, HW))

    nc = _get_program()

    in_maps = []
    for i in range(NCORES):
        in_maps.append(
            {
                "x": x3[i * BPC : (i + 1) * BPC],
                "r": x4[i * BPC : (i + 1) * BPC],
                "wT": wT,
                "bvec": bvec,
            }
        )

    trace = bool(int(os.environ.get("KERNEL_TRACE", "0")))
    kwargs = {}
    if os.environ.get("KERNEL_TMPDIR"):
        kwargs["tmpdir"] = os.environ["KERNEL_TMPDIR"]
    res = bass_utils.run_bass_kernel_spmd(
        nc, in_maps, core_ids=list(range(NCORES)), trace=trace, **kwargs
    )
    LAST_EXEC_TIME_NS = res.exec_time_ns
    _CACHE["last_results"] = res

    out = np.empty((B, C, HW), np.float32)
    for i in range(NCORES):
        out[i * BPC : (i + 1) * BPC] = res.results[i]["out"]
    return out.reshape(B, C, H, W)


# revision 6
# speedup vs baseline: 1.0469x; 1.0469x over previous
"""Trainium2 kernel for nn_MultiHeadCrossAttention_70952859730436.

The reference computes
    q = Wq x1 + bq ; k = Wk x2 + bk ; v = Wv x3 + bv          (1x1 convs)
    qk = q k^T / sqrt(32) ; attn = softmax(qk, -1)
    out_v = einsum('bhqk,bhvd->bhqd', attn, v)                 # sums over BOTH k and v
    out = Wo out_v + bo + x4

The einsum contracts `k` and `v` independently, so
    out_v[b,h,q,d] = (sum_k attn[b,h,q,k]) * (sum_v v[b,h,v,d]) = 1 * sum_v v[b,h,v,d]
(softmax rows sum to one).  The attention block therefore collapses to a
per-head channel sum of v, broadcast back to all 32 channels of the head:
    out = Wo (S (Wv x3 + bv)) + bo + x4 = (Wo S Wv) x3 + (Wo S bv + bo) + x4
with S the block matrix S[i,j] = [i//32 == j//32].  q, k, x1, x2 are unused.

The device kernel computes out = W_eff @ x3 + b_eff + x4 per spatial position:
a [256,256] x [256, B*H*W] matmul + bias + residual, data-parallel over batch
(2 batches per core across 8 cores).  The bias is folded into the matmul as a
K=1 accumulating matmul against a constant ones row, so the epilogue is a
single DVE add (psum + x4 tile).  Matmuls run as float32r (full-rate fp32
variant) unless KERNEL_MM_DTYPE=f32.
"""

import os

import numpy as np

B, C, H, W = 16, 256, 64, 64
HW = H * W  # 4096
RD, HEADS = 256, 8
RDH = RD // HEADS  # 32
NCORES = 8
BPC = B // NCORES  # batches per core = 2

CT = 1024  # spatial column tile per load DMA (4 KiB per partition)
NCT = HW // CT  # column tiles per batch
PB = 512  # psum free-dim chunk (one PSUM bank at fp32)
NPB = CT // PB
BUFS = 4

_CACHE = {}
_ONES = np.ones((1, PB), np.float32)

# Populated with the NEFF execution time (ns) of the slowest core after each
# traced run; test.py reads this.
LAST_EXEC_TIME_NS = None


def _build_program():
    import concourse.mybir as mybir
    import concourse.tile as tile
    from concourse import bacc

    nc = bacc.Bacc(
        "TRN2",
        target_bir_lowering=False,
        debug=False,
        num_devices=NCORES,
    )
    f32 = mybir.dt.float32
    if os.environ.get("KERNEL_MM_DTYPE", "f32r") == "f32r":
        mm_dt = mybir.dt.float32r
    else:
        mm_dt = f32

    # Tensors feeding the PE are declared float32r end-to-end (same bits as
    # f32; the PE's single-pass fp32 mode).  x4 / out stay plain f32.
    x = nc.dram_tensor("x", (BPC, C, HW), mm_dt, kind="ExternalInput").ap()
    r = nc.dram_tensor("r", (BPC, C, HW), f32, kind="ExternalInput").ap()
    wT = nc.dram_tensor("wT", (C, C), mm_dt, kind="ExternalInput").ap()  # [c_in, c_out]
    bvec = nc.dram_tensor("bvec", (2, 128), mm_dt, kind="ExternalInput").ap()
    ones_in = nc.dram_tensor("ones", (1, PB), mm_dt, kind="ExternalInput").ap()
    out = nc.dram_tensor("out", (BPC, C, HW), f32, kind="ExternalOutput").ap()

    with tile.TileContext(nc) as tc:
        with (
            tc.tile_pool(name="const", bufs=1) as cpool,
            tc.tile_pool(name="io", bufs=BUFS) as iopool,
            tc.tile_pool(name="ps", bufs=8, space="PSUM") as pspool,
        ):
            # Stationary weights: lhsT[k][m] = W_eff.T[k-block, m-block]
            w_tiles = {}
            for k in range(2):
                for m in range(2):
                    wt = cpool.tile([128, 128], mm_dt, name=f"w{k}{m}")
                    nc.sync.dma_start(
                        wt[:], wT[k * 128 : (k + 1) * 128, m * 128 : (m + 1) * 128]
                    )
                    w_tiles[k, m] = wt
            # Bias rows for the K=1 bias matmul: lhsT [1, 128] per m-block.
            b_tiles = []
            for m in range(2):
                bt = cpool.tile([1, 128], mm_dt, name=f"b{m}")
                nc.sync.dma_start(bt[:], bvec[m : m + 1, :])
                b_tiles.append(bt)
            ones = cpool.tile([1, PB], mm_dt, name="ones")
            nc.sync.dma_start(ones[:], ones_in[:])

            for b in range(BPC):
                for ct in range(NCT):
                    csl = slice(ct * CT, (ct + 1) * CT)
                    xts = []
                    for k in range(2):
                        xt = iopool.tile([128, CT], mm_dt, name=f"x{k}")
                        nc.sync.dma_start(xt[:], x[b, k * 128 : (k + 1) * 128, csl])
                        xts.append(xt)
                    rts = []
                    for m in range(2):
                        rt = iopool.tile([128, CT], f32, name=f"r{m}")
                        nc.sync.dma_start(rt[:], r[b, m * 128 : (m + 1) * 128, csl])
                        rts.append(rt)
                    for m in range(2):
                        ot = iopool.tile([128, CT], f32, name=f"o{m}")
                        for nb in range(NPB):
                            nsl = slice(nb * PB, (nb + 1) * PB)
                            ps = pspool.tile([128, PB], f32, name="ps")
                            for k in range(2):
                                nc.tensor.matmul(
                                    ps[:],
                                    w_tiles[k, m][:],
                                    xts[k][:, nsl],
                                    start=(k == 0),
                                    stop=False,
                                )
                            nc.tensor.matmul(
                                ps[:],
                                b_tiles[m][:],
                                ones[:],
                                start=False,
                                stop=True,
                            )
                            # epilogue: out = psum + x4 tile (single DVE pass)
                            nc.vector.tensor_add(ot[:, nsl], ps[:], rts[m][:, nsl])
                        # store from the scalar-engine HWDGE ring so loads
                        # (sync ring) and stores don't share a FIFO
                        nc.scalar.dma_start(
                            out[b, m * 128 : (m + 1) * 128, csl], ot[:]
                        )

    nc.compile()
    return nc


def _get_program():
    if "nc" not in _CACHE:
        _CACHE["nc"] = _build_program()
    return _CACHE["nc"]


def _fuse_weights(Wv, bv, Wo, bo):
    # W_eff = Wo @ S @ Wv,  b_eff = Wo @ S @ bv + bo  (S = per-head broadcast-sum)
    Wv = np.asarray(Wv, np.float64)
    bv = np.asarray(bv, np.float64)
    Wo = np.asarray(Wo, np.float64)
    bo = np.asarray(bo, np.float64)
    head_sum_W = Wv.reshape(HEADS, RDH, C).sum(axis=1)  # [8, 256]
    SWv = np.repeat(head_sum_W, RDH, axis=0)  # [256, 256]
    head_sum_b = bv.reshape(HEADS, RDH).sum(axis=1)  # [8]
    Sbv = np.repeat(head_sum_b, RDH)  # [256]
    W_eff = Wo @ SWv  # [C, C]
    b_eff = Wo @ Sbv + bo  # [C]
    return W_eff.astype(np.float32), b_eff.astype(np.float32)


def kernel(x1, x2, x3, x4, Wq, bq, Wk, bk, Wv, bv, Wo, bo):
    global LAST_EXEC_TIME_NS
    from concourse import bass_utils

    W_eff, b_eff = _fuse_weights(Wv, bv, Wo, bo)
    wT = np.ascontiguousarray(W_eff.T)  # [c_in, c_out]
    bvec = np.ascontiguousarray(b_eff.reshape(2, 128))

    x3 = np.ascontiguousarray(np.asarray(x3, np.float32).reshape(B, C, HW))
    x4 = np.ascontiguousarray(np.asarray(x4, np.float32).reshape(B, C, HW))

    nc = _get_program()

    in_maps = []
    for i in range(NCORES):
        in_maps.append(
            {
                "x": x3[i * BPC : (i + 1) * BPC],
                "r": x4[i * BPC : (i + 1) * BPC],
                "wT": wT,
                "bvec": bvec,
                "ones": _ONES,
            }
        )

    trace = bool(int(os.environ.get("KERNEL_TRACE", "0")))
    kwargs = {}
    if os.environ.get("KERNEL_TMPDIR"):
        kwargs["tmpdir"] = os.environ["KERNEL_TMPDIR"]
    res = bass_utils.run_bass_kernel_spmd(
        nc, in_maps, core_ids=list(range(NCORES)), trace=trace, **kwargs
    )
    LAST_EXEC_TIME_NS = res.exec_time_ns
    _CACHE["last_results"] = res

    out = np.empty((B, C, HW), np.float32)
    for i in range(NCORES):
        out[i * BPC : (i + 1) * BPC] = res.results[i]["out"]
    return out.reshape(B, C, H, W)


# revision 7
# speedup vs baseline: 1.1600x; 1.1081x over previous
"""Trainium2 kernel for nn_MultiHeadCrossAttention_70952859730436.

The reference computes
    q = Wq x1 + bq ; k = Wk x2 + bk ; v = Wv x3 + bv          (1x1 convs)
    qk = q k^T / sqrt(32) ; attn = softmax(qk, -1)
    out_v = einsum('bhqk,bhvd->bhqd', attn, v)                 # sums over BOTH k and v
    out = Wo out_v + bo + x4

The einsum contracts `k` and `v` independently, so
    out_v[b,h,q,d] = (sum_k attn[b,h,q,k]) * (sum_v v[b,h,v,d]) = 1 * sum_v v[b,h,v,d]
(softmax rows sum to one).  The attention block therefore collapses to a
per-head channel sum of v, broadcast back to all 32 channels of the head:
    out = Wo (S (Wv x3 + bv)) + bo + x4 = (Wo S Wv) x3 + (Wo S bv + bo) + x4
with S the block matrix S[i,j] = [i//32 == j//32].  q, k, x1, x2 are unused.

The device kernel computes out = W_eff @ x3 + b_eff + x4 per spatial position:
a [256,256] x [256, B*H*W] matmul + bias + residual, data-parallel over batch
(2 batches per core across 8 cores).  The bias is folded into the matmul as a
K=1 accumulating matmul against a constant ones row, so the epilogue is a
single DVE add (psum + x4 tile).  Matmuls run as float32r (full-rate fp32
variant) unless KERNEL_MM_DTYPE=f32.
"""

import os

import numpy as np

B, C, H, W = 16, 256, 64, 64
HW = H * W  # 4096
RD, HEADS = 256, 8
RDH = RD // HEADS  # 32
NCORES = 8
BPC = B // NCORES  # batches per core = 2

CT = 1024  # spatial column tile per load DMA (4 KiB per partition)
NCT = HW // CT  # column tiles per batch
PB = 512  # psum free-dim chunk (one PSUM bank at fp32)
NPB = CT // PB
BUFS = 6

_CACHE = {}

# Populated with the NEFF execution time (ns) of the slowest core after each
# traced run; test.py reads this.
LAST_EXEC_TIME_NS = None


def _build_program():
    import concourse.mybir as mybir
    import concourse.tile as tile
    from concourse import bacc

    nc = bacc.Bacc(
        "TRN2",
        target_bir_lowering=False,
        debug=False,
        num_devices=NCORES,
    )
    f32 = mybir.dt.float32
    if os.environ.get("KERNEL_MM_DTYPE", "f32r") == "f32r":
        mm_dt = mybir.dt.float32r
    else:
        mm_dt = f32

    # Tensors feeding the PE are declared float32r end-to-end (same bits as
    # f32; the PE's single-pass fp32 mode).  x4 / out stay plain f32.
    x = nc.dram_tensor("x", (BPC, C, HW), mm_dt, kind="ExternalInput").ap()
    r = nc.dram_tensor("r", (BPC, C, HW), f32, kind="ExternalInput").ap()
    wT = nc.dram_tensor("wT", (C, C), mm_dt, kind="ExternalInput").ap()  # [c_in, c_out]
    bvec = nc.dram_tensor("bvec", (C, 1), f32, kind="ExternalInput").ap()
    out = nc.dram_tensor("out", (BPC, C, HW), f32, kind="ExternalOutput").ap()

    with tile.TileContext(nc) as tc:
        with (
            tc.tile_pool(name="const", bufs=1) as cpool,
            tc.tile_pool(name="io", bufs=BUFS) as iopool,
            tc.tile_pool(name="ps", bufs=8, space="PSUM") as pspool,
        ):
            # Stationary weights: lhsT[k][m] = W_eff.T[k-block, m-block]
            w_tiles = {}
            for k in range(2):
                for m in range(2):
                    wt = cpool.tile([128, 128], mm_dt, name=f"w{k}{m}")
                    nc.sync.dma_start(
                        wt[:], wT[k * 128 : (k + 1) * 128, m * 128 : (m + 1) * 128]
                    )
                    w_tiles[k, m] = wt
            # Per-partition bias columns [128,1] per m-block (added to x4 tiles).
            b_tiles = []
            for m in range(2):
                bt = cpool.tile([128, 1], f32, name=f"b{m}")
                nc.sync.dma_start(bt[:], bvec[m * 128 : (m + 1) * 128, :])
                b_tiles.append(bt)

            for b in range(BPC):
                for ct in range(NCT):
                    csl = slice(ct * CT, (ct + 1) * CT)
                    xts = []
                    for k in range(2):
                        xt = iopool.tile([128, CT], mm_dt, name=f"x{k}")
                        nc.sync.dma_start(xt[:], x[b, k * 128 : (k + 1) * 128, csl])
                        xts.append(xt)
                    rts = []
                    for m in range(2):
                        rt = iopool.tile([128, CT], f32, name=f"r{m}")
                        nc.scalar.dma_start(rt[:], r[b, m * 128 : (m + 1) * 128, csl])
                        # fold the channel bias into the residual tile
                        nc.vector.tensor_scalar_add(rt[:], rt[:], b_tiles[m][:])
                        rts.append(rt)
                    for m in range(2):
                        ot = iopool.tile([128, CT], f32, name=f"o{m}")
                        for nb in range(NPB):
                            nsl = slice(nb * PB, (nb + 1) * PB)
                            ps = pspool.tile([128, PB], f32, name="ps")
                            for k in range(2):
                                nc.tensor.matmul(
                                    ps[:],
                                    w_tiles[k, m][:],
                                    xts[k][:, nsl],
                                    start=(k == 0),
                                    stop=(k == 1),
                                )
                            # epilogue: out = psum + x4 tile (single DVE pass)
                            nc.vector.tensor_add(ot[:, nsl], ps[:], rts[m][:, nsl])
                        # alternate stores across the two HWDGE rings
                        eng = nc.sync if m == 0 else nc.scalar
                        eng.dma_start(out[b, m * 128 : (m + 1) * 128, csl], ot[:])

    nc.compile()
    return nc


def _get_program():
    if "nc" not in _CACHE:
        _CACHE["nc"] = _build_program()
    return _CACHE["nc"]


def _fuse_weights(Wv, bv, Wo, bo):
    # W_eff = Wo @ S @ Wv,  b_eff = Wo @ S @ bv + bo  (S = per-head broadcast-sum)
    Wv = np.asarray(Wv, np.float64)
    bv = np.asarray(bv, np.float64)
    Wo = np.asarray(Wo, np.float64)
    bo = np.asarray(bo, np.float64)
    head_sum_W = Wv.reshape(HEADS, RDH, C).sum(axis=1)  # [8, 256]
    SWv = np.repeat(head_sum_W, RDH, axis=0)  # [256, 256]
    head_sum_b = bv.reshape(HEADS, RDH).sum(axis=1)  # [8]
    Sbv = np.repeat(head_sum_b, RDH)  # [256]
    W_eff = Wo @ SWv  # [C, C]
    b_eff = Wo @ Sbv + bo  # [C]
    return W_eff.astype(np.float32), b_eff.astype(np.float32)


def kernel(x1, x2, x3, x4, Wq, bq, Wk, bk, Wv, bv, Wo, bo):
    global LAST_EXEC_TIME_NS
    from concourse import bass_utils

    W_eff, b_eff = _fuse_weights(Wv, bv, Wo, bo)
    wT = np.ascontiguousarray(W_eff.T)  # [c_in, c_out]
    bvec = np.ascontiguousarray(b_eff.reshape(C, 1))

    x3 = np.ascontiguousarray(np.asarray(x3, np.float32).reshape(B, C, HW))
    x4 = np.ascontiguousarray(np.asarray(x4, np.float32).reshape(B, C, HW))

    nc = _get_program()

    in_maps = []
    for i in range(NCORES):
        in_maps.append(
            {
                "x": x3[i * BPC : (i + 1) * BPC],
                "r": x4[i * BPC : (i + 1) * BPC],
                "wT": wT,
                "bvec": bvec,
            }
        )

    trace = bool(int(os.environ.get("KERNEL_TRACE", "0")))
    kwargs = {}
    if os.environ.get("KERNEL_TMPDIR"):
        kwargs["tmpdir"] = os.environ["KERNEL_TMPDIR"]
    res = bass_utils.run_bass_kernel_spmd(
        nc, in_maps, core_ids=list(range(NCORES)), trace=trace, **kwargs
    )
    LAST_EXEC_TIME_NS = res.exec_time_ns
    _CACHE["last_results"] = res

    out = np.empty((B, C, HW), np.float32)
    for i in range(NCORES):
        out[i * BPC : (i + 1) * BPC] = res.results[i]["out"]
    return out.reshape(B, C, H, W)
